# revision 1
# baseline (speedup 1.0000x reference)
"""Bass/Tile TRN2 kernel: adaptive min 2D pooling (8x8 grid) of [B,512,512] f32.

Full input [128, 512, 512] f32 -> output [128, 64] f32.
Data parallel over 8 NeuronCores: 16 matrices per core.

Per-core algorithm (x_local [16, 512, 512] -> y_local [16, 64]):
  1. For each matrix m: DMA [512,512] into SBUF as [128, 2048] with
     partition p = row within a 128-row quarter, free = (q, c):
     row = q*128 + p. Each partition line is 4 chunks of 2KB contiguous
     HBM -> near line-rate DMA.
  2. DVE reduce_min over the innermost 64-column groups:
     [128, (q gc c=64)] -> acc[:, m*32 + (q*8+gc)]  (min over c).
     acc is [128, 512] with free = (m, q, gc), partition = row-in-quarter.
  3. Cross-partition min (over the 128 rows-in-quarter = 2 bands x 64 rows)
     cannot run on DVE -> PE-transpose each 128-col block of acc into PSUM
     (exact data movement), then DVE reduce_min over the row halves:
     accT_k [128=(m' q gc), (b=2, r=64)] -> res[:, 2k+b].
  4. Second PE transpose turns res [128, (k b)] into resT [(k b), (mp q gc)]
     (via PSUM + DVE copy), so the 8 output DMAs read contiguous 512B SBUF
     lines and write 32B-contiguous runs into y [16, 64].
"""

import threading

import numpy as np

B, N, M = 128, 512, 512
GRID = 8
NCORES = 8
BL = B // NCORES  # 16 matrices per core

_lock = threading.RLock()
_cache: dict = {}


def _build(n_iters: int = 1, bufs: int = 6):
    import concourse.bacc as bacc
    import concourse.mybir as mybir
    import concourse.tile as tile

    f32 = mybir.dt.float32

    nc = bacc.Bacc("TRN2", target_bir_lowering=False, debug=False)
    x = nc.dram_tensor("x", [BL, N, M], f32, kind="ExternalInput").ap()
    y = nc.dram_tensor("y", [BL, GRID * GRID], f32, kind="ExternalOutput").ap()

    with tile.TileContext(nc) as tc:
        with (
            tc.tile_pool(name="inp", bufs=bufs) as inp,
            tc.tile_pool(name="accp", bufs=2) as accp,
            tc.tile_pool(name="resp", bufs=2) as resp,
            tc.tile_pool(name="rtp", bufs=2) as rtp,
            tc.tile_pool(name="idp", bufs=1) as idp,
            tc.tile_pool(name="psw", bufs=2, space="PSUM") as psw,
            tc.tile_pool(name="ps2", bufs=2, space="PSUM") as ps2,
        ):
            # identity matrix for the PE transpose
            ones = idp.tile([128, 128], f32)
            ident = idp.tile([128, 128], f32)
            nc.gpsimd.memset(ones[:], 1.0)
            nc.gpsimd.affine_select(
                ident[:],
                ones[:],
                pattern=[[-1, 128]],
                compare_op=mybir.AluOpType.is_equal,
                fill=0.0,
                base=0,
                channel_multiplier=1,
            )

            for _ in range(n_iters):  # n_iters>1 only for benchmarking
                acc = accp.tile([128, 512], f32)
                res = resp.tile([128, 8], f32)

                # stage 1: per-matrix load + min over column groups.
                # Alternate the two HWDGE rings (SP / ACT) — a single ring
                # serializes with ~1us bubbles between DMAs (300 GB/s); the
                # two rings together reach the HBM roofline (~373 GB/s).
                # The last matrix is split into 4 quarter DMAs/reduces so the
                # kernel tail only waits on a 256KB transfer + small reduce
                # (-10us measured). Interleaving stage 2 into this stream was
                # measured SLOWER on HW (+12-30us; PE/PSUM traffic and scatter
                # DMAs stall the FIFO DMA rings mid-stream), so stage 2 stays
                # at the end.
                for m in range(BL):
                    t = inp.tile([128, 4 * M], f32)
                    if m == BL - 1:
                        for q in range(4):
                            eng = nc.sync if q % 2 == 0 else nc.scalar
                            eng.dma_start(
                                t[:, q * M : (q + 1) * M],
                                x[m, q * 128 : (q + 1) * 128, :],
                            )
                            nc.vector.tensor_reduce(
                                acc[:, m * 32 + q * 8 : m * 32 + (q + 1) * 8],
                                t[:, q * M : (q + 1) * M].rearrange(
                                    "p (g c) -> p g c", c=M // GRID
                                ),
                                axis=mybir.AxisListType.X,
                                op=mybir.AluOpType.min,
                            )
                    else:
                        eng = nc.sync if m % 2 == 0 else nc.scalar
                        eng.dma_start(
                            t[:].rearrange("p (q c) -> p q c", q=4),
                            x[m].rearrange("(q p) c -> p q c", p=128),
                        )
                        nc.vector.tensor_reduce(
                            acc[:, m * 32 : (m + 1) * 32],
                            t[:].rearrange("p (g c) -> p g c", c=M // GRID),
                            axis=mybir.AxisListType.X,
                            op=mybir.AluOpType.min,
                        )

                # stage 2: cross-partition min via PE transpose + free-dim
                # reduce over the row halves (bands). All 4 transposes land
                # in one PSUM-bank tile so a single fused reduce replaces 4.
                pt_all = psw.tile([128, 512], f32)
                for k in range(4):
                    nc.tensor.transpose(
                        pt_all[:, k * 128 : (k + 1) * 128],
                        acc[:, k * 128 : (k + 1) * 128],
                        ident[:],
                    )
                nc.vector.tensor_reduce(
                    res[:],
                    pt_all[:].rearrange("p (g r) -> p g r", r=64),
                    axis=mybir.AxisListType.X,
                    op=mybir.AluOpType.min,
                )

                # Second PE transpose: res [128,(k b)] -> resT [(k b), (mp q gc)]
                # so each output DMA reads one 512B-contiguous SBUF line and
                # writes 16 x 32B runs, instead of 128 x 4B scattered writes
                # (-9us/iter measured head-to-head).
                pt2 = ps2.tile([128, 128], f32)
                nc.tensor.transpose(pt2[0:8, :], res[:], ident[:])
                resT = rtp.tile([128, 128], f32)
                # ACT does the PSUM->SBUF copy so it overlaps DVE tail work
                nc.scalar.copy(resT[0:8, :], pt2[0:8, :])

                # resT[(k b), (mp q gc)] -> y[4k+mp, (2q+b)*8+gc]
                for k in range(4):
                    for b in range(2):
                        eng = nc.sync if (2 * k + b) % 2 == 0 else nc.scalar
                        eng.dma_start(
                            y[4 * k : 4 * (k + 1)].rearrange(
                                "mp (q b gc) -> mp q b gc", q=4, b=2
                            )[:, :, b, :],
                            resT[2 * k + b : 2 * k + b + 1, :],
                        )

    nc.compile()
    return nc


def _get_nc():
    with _lock:
        if "nc" not in _cache:
            _cache["nc"] = _build()
        return _cache["nc"]


def _get_runner():
    """Build the shard_map-jitted 8-core runner ONCE and reuse it across
    kernel() calls (run_bass_kernel_spmd re-jits per call, ~seconds of host
    overhead). Mirrors bass2jax.run_bass_via_pjrt's multi-core wiring."""
    if "runner" in _cache:
        return _cache["runner"]

    import jax
    from jax.sharding import Mesh, PartitionSpec
    from jax.experimental.shard_map import shard_map

    from concourse import bass2jax, mybir

    nc = _get_nc()
    bass2jax.install_neuronx_cc_hook()

    partition_name = nc.partition_id_tensor.name if nc.partition_id_tensor else None
    in_names, out_names, out_avals = [], [], []
    for alloc in nc.m.functions[0].allocations:
        if not isinstance(alloc, mybir.MemoryLocationSet):
            continue
        name = alloc.memorylocations[0].name
        if alloc.kind == "ExternalInput":
            if name != partition_name:
                in_names.append(name)
        elif alloc.kind == "ExternalOutput":
            out_names.append(name)
            out_avals.append(
                jax.core.ShapedArray(
                    tuple(alloc.tensor_shape), mybir.dt.np(alloc.dtype)
                )
            )
    assert in_names == ["x"] and out_names == ["y"]
    all_in_names = list(in_names) + list(out_names)
    if partition_name is not None:
        all_in_names.append(partition_name)

    def _body(*args):
        operands = list(args)
        if partition_name is not None:
            operands.append(bass2jax.partition_id_tensor())
        outs = bass2jax._bass_exec_p.bind(
            *operands,
            out_avals=tuple(out_avals),
            in_names=tuple(all_in_names),
            out_names=tuple(out_names),
            lowering_input_output_aliases=(),
            sim_require_finite=True,
            sim_require_nnan=True,
            nc=nc,
        )
        return tuple(outs)

    devices = jax.devices()[:NCORES]
    mesh = Mesh(np.asarray(devices), ("core",))
    sharded = jax.jit(
        shard_map(
            _body,
            mesh=mesh,
            in_specs=(PartitionSpec("core"),) * 2,
            out_specs=(PartitionSpec("core"),),
            check_rep=False,
        ),
        donate_argnums=(1,),
        keep_unused=True,
    )
    _cache["runner"] = sharded
    return sharded


def _kernel_fallback(xs: np.ndarray) -> np.ndarray:
    from concourse.bass_utils import run_bass_kernel_spmd

    nc = _get_nc()
    in_maps = [{"x": xs[i * BL : (i + 1) * BL]} for i in range(NCORES)]
    r = run_bass_kernel_spmd(nc, in_maps, list(range(NCORES)))
    return np.concatenate([r.results[i]["y"] for i in range(NCORES)], axis=0)


def kernel(sim_matrices: np.ndarray) -> np.ndarray:
    xs = np.ascontiguousarray(sim_matrices, dtype=np.float32)
    assert xs.shape == (B, N, M), xs.shape
    with _lock:
        try:
            runner = _get_runner()
            zeros = np.zeros((B, GRID * GRID), np.float32)
            (y_global,) = runner(xs, zeros)
            return np.asarray(y_global)
        except Exception:
            return _kernel_fallback(xs)



# revision 15
# speedup vs baseline: 2.0354x; 2.0354x over previous
"""Bass/Tile TRN2 kernel: adaptive min 2D pooling (8x8 grid) of [B,512,512] f32.

Full input [128, 512, 512] f32 -> output [128, 64] f32.
Data parallel over 8 NeuronCores: 16 matrices per core. The kernel is
DMA-bandwidth-bound (16.78 MB HBM reads per core at ~360 GB/s ~= 46 us);
everything else is structured to keep the two HWDGE input rings streaming
back-to-back with zero stalls:

  - "quad" layout (default, see _emit_quad_iter): partition p holds 4
    consecutive matrix rows, so every input DMA descriptor is one 8KB
    contiguous HBM run (128 descriptors/matrix), alternating the two
    HWDGE rings (sync/scalar) per matrix.
  - deep input tile pool (bufs=10) keeps ~10 x 1MB loads in flight so the
    DMA engines never starve while compute tails drain.
  - the single output DMA runs on the Pool-engine SWDGE path; the HWDGE
    rings are in-order FIFOs, so an output DMA (which depends on the full
    compute tail) queued there would stall the next iteration's input
    stream behind it.
  - cross-partition mins use 2 PE transposes through PSUM; DVE does all
    free-dim reductions (~2x slack vs DMA).

The older "qrow" layout (partition = row-in-quarter, 4x2KB descriptor
chunks, 5 transposes, 8 output DMAs) is kept for reference/benchmarking.
"""

import threading

import numpy as np

B, N, M = 128, 512, 512
GRID = 8
NCORES = 8
BL = B // NCORES  # 16 matrices per core

_lock = threading.RLock()
_cache: dict = {}


def _build(
    n_iters: int = 1,
    bufs: int = 10,
    out_on_rings: bool = False,
    layout: str = "quad",
    pack: int = 1,
    n_in_qs: int = 2,
):
    import concourse.bacc as bacc
    import concourse.mybir as mybir
    import concourse.tile as tile

    f32 = mybir.dt.float32

    nc = bacc.Bacc("TRN2", target_bir_lowering=False, debug=False)
    x = nc.dram_tensor("x", [BL, N, M], f32, kind="ExternalInput").ap()
    y = nc.dram_tensor("y", [BL, GRID * GRID], f32, kind="ExternalOutput").ap()

    with tile.TileContext(nc) as tc:
        with (
            tc.tile_pool(name="inp", bufs=bufs) as inp,
            tc.tile_pool(name="accp", bufs=2) as accp,
            tc.tile_pool(name="resp", bufs=2) as resp,
            tc.tile_pool(name="rtp", bufs=2) as rtp,
            tc.tile_pool(name="idp", bufs=1) as idp,
            tc.tile_pool(name="psw", bufs=2, space="PSUM") as psw,
            tc.tile_pool(name="ps2", bufs=2, space="PSUM") as ps2,
        ):
            # identity matrix for the PE transpose
            ones = idp.tile([128, 128], f32)
            ident = idp.tile([128, 128], f32)
            nc.gpsimd.memset(ones[:], 1.0)
            nc.gpsimd.affine_select(
                ident[:],
                ones[:],
                pattern=[[-1, 128]],
                compare_op=mybir.AluOpType.is_equal,
                fill=0.0,
                base=0,
                channel_multiplier=1,
            )

            for _ in range(n_iters):  # n_iters>1 only for benchmarking
                if layout == "quad":
                    _emit_quad_iter(
                        nc, mybir, x, y, inp, accp, resp, rtp, psw, ps2,
                        ident, out_on_rings, pack, n_in_qs,
                    )
                    continue
                acc = accp.tile([128, 512], f32)
                res = resp.tile([128, 8], f32)

                # stage 1: per-matrix load + min over column groups.
                # Alternate the two HWDGE rings (SP / ACT) — a single ring
                # serializes with ~1us bubbles between DMAs (300 GB/s); the
                # two rings together reach the HBM roofline (~373 GB/s).
                # The last matrix is split into 4 quarter DMAs/reduces so the
                # kernel tail only waits on a 256KB transfer + small reduce
                # (-10us measured). Interleaving stage 2 into this stream was
                # measured SLOWER on HW (+12-30us; PE/PSUM traffic and scatter
                # DMAs stall the FIFO DMA rings mid-stream), so stage 2 stays
                # at the end.
                for m in range(BL):
                    t = inp.tile([128, 4 * M], f32)
                    if m == BL - 1:
                        for q in range(4):
                            eng = nc.sync if q % 2 == 0 else nc.scalar
                            eng.dma_start(
                                t[:, q * M : (q + 1) * M],
                                x[m, q * 128 : (q + 1) * 128, :],
                            )
                            nc.vector.tensor_reduce(
                                acc[:, m * 32 + q * 8 : m * 32 + (q + 1) * 8],
                                t[:, q * M : (q + 1) * M].rearrange(
                                    "p (g c) -> p g c", c=M // GRID
                                ),
                                axis=mybir.AxisListType.X,
                                op=mybir.AluOpType.min,
                            )
                    else:
                        eng = nc.sync if m % 2 == 0 else nc.scalar
                        eng.dma_start(
                            t[:].rearrange("p (q c) -> p q c", q=4),
                            x[m].rearrange("(q p) c -> p q c", p=128),
                        )
                        nc.vector.tensor_reduce(
                            acc[:, m * 32 : (m + 1) * 32],
                            t[:].rearrange("p (g c) -> p g c", c=M // GRID),
                            axis=mybir.AxisListType.X,
                            op=mybir.AluOpType.min,
                        )

                # stage 2: cross-partition min via PE transpose + free-dim
                # reduce over the row halves (bands). All 4 transposes land
                # in one PSUM-bank tile so a single fused reduce replaces 4.
                pt_all = psw.tile([128, 512], f32)
                for k in range(4):
                    nc.tensor.transpose(
                        pt_all[:, k * 128 : (k + 1) * 128],
                        acc[:, k * 128 : (k + 1) * 128],
                        ident[:],
                    )
                nc.vector.tensor_reduce(
                    res[:],
                    pt_all[:].rearrange("p (g r) -> p g r", r=64),
                    axis=mybir.AxisListType.X,
                    op=mybir.AluOpType.min,
                )

                # Second PE transpose: res [128,(k b)] -> resT [(k b), (mp q gc)]
                # so each output DMA reads one 512B-contiguous SBUF line and
                # writes 16 x 32B runs, instead of 128 x 4B scattered writes
                # (-9us/iter measured head-to-head).
                pt2 = ps2.tile([128, 128], f32)
                nc.tensor.transpose(pt2[0:8, :], res[:], ident[:])
                resT = rtp.tile([128, 128], f32)
                # ACT does the PSUM->SBUF copy so it overlaps DVE tail work
                nc.scalar.copy(resT[0:8, :], pt2[0:8, :])

                # resT[(k b), (mp q gc)] -> y[4k+mp, (2q+b)*8+gc]
                # Output DMAs go via the Pool-engine SWDGE path: the HWDGE
                # rings are in-order FIFOs, so putting these (which depend on
                # the full compute tail) on sync/scalar would stall the next
                # iteration's input stream behind them.
                for k in range(4):
                    for b in range(2):
                        if out_on_rings:
                            eng = nc.sync if (2 * k + b) % 2 == 0 else nc.scalar
                        else:
                            eng = nc.gpsimd
                        eng.dma_start(
                            y[4 * k : 4 * (k + 1)].rearrange(
                                "mp (q b gc) -> mp q b gc", q=4, b=2
                            )[:, :, b, :],
                            resT[2 * k + b : 2 * k + b + 1, :],
                        )

    nc.compile()
    return nc


def _emit_quad_iter(
    nc, mybir, x, y, inp, accp, resp, rtp, psw, ps2, ident, out_on_rings, pack=1,
    n_in_qs=2,
):
    """One pooling iteration with 8KB-contiguous partition lines.

    Layout: partition p holds 4 consecutive matrix rows (4p..4p+3), so each
    input DMA descriptor is one 8KB contiguous HBM run (128 descs/matrix vs
    512 for the qrow layout).  All 4 rows of a partition belong to the same
    64-row output group (group = p//16), so:
      stage 1 (per matrix): reduce innermost 64 cols:
        t[p, (rr g gc)] -> u[p, m*32 + (rr*8+g)]
      stage 2: strided reduce over rr: u[p, (m rr g)] -> v[p, (m g)]
      stage 3: PE-transpose v -> pt[(m g), p]; reduce 16-partition bands:
        pt[(m g), (rg pp)] -> res[(m g), rg]
      stage 4: PE-transpose res -> resT[rg, (m g)]; 8 output DMAs
        y[m, rg*8+g] <- resT[rg, m*8+g].
    """
    f32 = mybir.dt.float32
    in_engines = [nc.sync, nc.scalar, nc.gpsimd][:n_in_qs]
    u = accp.tile([128, 512], f32)
    v = resp.tile([128, 128], f32)
    for m in range(0, BL, pack):
        t = inp.tile([128, pack * 4 * M], f32)
        eng = in_engines[(m // pack) % len(in_engines)]
        eng.dma_start(
            t[:].rearrange("p (mm rc) -> p mm rc", mm=pack),
            x[m : m + pack].rearrange("mm (p rr) c -> p mm (rr c)", rr=4),
        )
        nc.vector.tensor_reduce(
            u[:, m * 32 : (m + pack) * 32],
            t[:].rearrange("p (rg c) -> p rg c", c=M // GRID),
            axis=mybir.AxisListType.X,
            op=mybir.AluOpType.min,
        )
    nc.vector.tensor_reduce(
        v[:].rearrange("p (m g) -> p m g", m=BL),
        u[:].rearrange("p (m rr g) -> p m g rr", m=BL, rr=4),
        axis=mybir.AxisListType.X,
        op=mybir.AluOpType.min,
    )
    pt = psw.tile([128, 128], f32)
    nc.tensor.transpose(pt[:], v[:], ident[:])
    res = resp.tile([128, 8], f32)
    nc.vector.tensor_reduce(
        res[:],
        pt[:].rearrange("P (rg pp) -> P rg pp", pp=16),
        axis=mybir.AxisListType.X,
        op=mybir.AluOpType.min,
    )
    pt2 = ps2.tile([128, 128], f32)
    nc.tensor.transpose(pt2[0:8, :], res[:], ident[:])
    resT = rtp.tile([128, 128], f32)
    nc.scalar.copy(resT[0:8, :], pt2[0:8, :])
    # Single output DMA: dst y viewed as [rg, m, g] (strides 32B/256B/4B,
    # 3-dim AP, contiguous inner) matches src resT[0:8,:] element order
    # (partition rg outer, free (m g) inner) exactly.
    out_eng = nc.sync if out_on_rings else nc.gpsimd
    out_eng.dma_start(
        y.rearrange("m (rg g) -> rg m g", rg=8),
        resT[0:8, :],
    )


def _get_nc():
    with _lock:
        if "nc" not in _cache:
            _cache["nc"] = _build()
        return _cache["nc"]


def _get_runner():
    """Build the shard_map-jitted 8-core runner ONCE and reuse it across
    kernel() calls (run_bass_kernel_spmd re-jits per call, ~seconds of host
    overhead). Mirrors bass2jax.run_bass_via_pjrt's multi-core wiring."""
    if "runner" in _cache:
        return _cache["runner"]

    import jax
    from jax.sharding import Mesh, PartitionSpec
    from jax.experimental.shard_map import shard_map

    from concourse import bass2jax, mybir

    nc = _get_nc()
    bass2jax.install_neuronx_cc_hook()

    partition_name = nc.partition_id_tensor.name if nc.partition_id_tensor else None
    in_names, out_names, out_avals = [], [], []
    for alloc in nc.m.functions[0].allocations:
        if not isinstance(alloc, mybir.MemoryLocationSet):
            continue
        name = alloc.memorylocations[0].name
        if alloc.kind == "ExternalInput":
            if name != partition_name:
                in_names.append(name)
        elif alloc.kind == "ExternalOutput":
            out_names.append(name)
            out_avals.append(
                jax.core.ShapedArray(
                    tuple(alloc.tensor_shape), mybir.dt.np(alloc.dtype)
                )
            )
    assert in_names == ["x"] and out_names == ["y"]
    all_in_names = list(in_names) + list(out_names)
    if partition_name is not None:
        all_in_names.append(partition_name)

    def _body(*args):
        operands = list(args)
        if partition_name is not None:
            operands.append(bass2jax.partition_id_tensor())
        outs = bass2jax._bass_exec_p.bind(
            *operands,
            out_avals=tuple(out_avals),
            in_names=tuple(all_in_names),
            out_names=tuple(out_names),
            lowering_input_output_aliases=(),
            sim_require_finite=True,
            sim_require_nnan=True,
            nc=nc,
        )
        return tuple(outs)

    devices = jax.devices()[:NCORES]
    mesh = Mesh(np.asarray(devices), ("core",))
    sharded = jax.jit(
        shard_map(
            _body,
            mesh=mesh,
            in_specs=(PartitionSpec("core"),) * 2,
            out_specs=(PartitionSpec("core"),),
            check_rep=False,
        ),
        donate_argnums=(1,),
        keep_unused=True,
    )
    _cache["runner"] = sharded
    return sharded


def _kernel_fallback(xs: np.ndarray) -> np.ndarray:
    from concourse.bass_utils import run_bass_kernel_spmd

    nc = _get_nc()
    in_maps = [{"x": xs[i * BL : (i + 1) * BL]} for i in range(NCORES)]
    r = run_bass_kernel_spmd(nc, in_maps, list(range(NCORES)))
    return np.concatenate([r.results[i]["y"] for i in range(NCORES)], axis=0)


def kernel(sim_matrices: np.ndarray) -> np.ndarray:
    xs = np.ascontiguousarray(sim_matrices, dtype=np.float32)
    assert xs.shape == (B, N, M), xs.shape
    with _lock:
        try:
            runner = _get_runner()
            zeros = np.zeros((B, GRID * GRID), np.float32)
            (y_global,) = runner(xs, zeros)
            return np.asarray(y_global)
        except Exception:
            return _kernel_fallback(xs)

